# revision 33
# baseline (speedup 1.0000x reference)
"""Causal multi-head attention on 8 Trainium2 NeuronCores.

Problem: B=2, S=4096, D_MODEL=768, H=12, D_HEAD=64, fp32 I/O.

Sharding: (batch, head-group) -> core.  Cores 0-3 take batch 0, cores 4-7
take batch 1; each core computes 3 of the 12 heads for its batch and emits a
partial output [S, D_MODEL] (its heads' contribution to the W_O contraction)
in bf16.  The host sums the 4 partials per batch and adds b_O.

v2 design (vs v1): the scalar engine's EXP throughput (1 elem/cycle/lane) is
the fundamental floor (~190us/core), so the kernel keeps ACT dense from t~5us
and strips everything else off it:
  1. QKV projections are interleaved INTO the flash window loop as PE filler
     thunks (proj chunk c+2 runs during window c), so exp starts immediately
     after chunk 0 is projected.  Projection weights are packed into 5 column
     groups [Q01|K01|V01|Q2K2|V2] so 4 of 5 matmuls use the full 128-wide PE.
     Epilogue copies run on DVE (not ACT).  K2's partition shift (rows 64-127
     -> 0-63) rides a small SBUF->SBUF DMA.
  2. V is transposed to [keys, z] layout by the DMA xbar (dma transpose),
     costing zero PE/DVE cycles; a ones column gives softmax row sums for
     free in the AV matmuls (PSUM row 64).
  3. Head2 exp is batched over 2 key-tiles per call ([128,1024] activations
     everywhere), halving ACT per-instruction overhead.
  4. O-projection packs heads 0,1 as one contraction-128 matmul: Zn for h1 is
     partition-shifted to rows 64-127 by DMA, so per row-tile the projection
     is 2 matmuls (h01 + h2) instead of 3.
  5. Output partials are written in bf16 (halves output DMA); the host sums
     the 4 partials per batch in fp32.
  6. Softmax row-sum normalization (DVE transpose + reciprocal) is batched
     across the 3 heads per window; the 1/r broadcast stays a rank-1 PE
     matmul emitted as a filler so the PE never stalls long enough to
     re-throttle its clock (HAM drops 2.4->1.2 GHz after ~3.4us idle).
"""

import numpy as np
import ml_dtypes

B, S, DM, H, DH = 2, 4096, 768, 12, 64
NCORES = 8
GROUPS = 4                  # head-groups per batch
HPC = H // GROUPS           # heads per core = 3
P = 128
QCH = 512                   # psum bank width (fp32)

_BF = ml_dtypes.bfloat16

_cache = {}
DEBUG_DUMPS = False


def _build(seq_len, use_biases):
    import concourse.bacc as bacc
    import concourse.mybir as mybir
    import concourse.tile as tile

    f32 = mybir.dt.float32
    f32r = mybir.dt.float32r
    bf16 = mybir.dt.bfloat16
    Exp = mybir.ActivationFunctionType.Exp
    mult = mybir.AluOpType.mult
    add = mybir.AluOpType.add

    SQ = seq_len
    n_kt = SQ // P               # k tiles
    n_tt = SQ // P               # output row tiles
    n_ch = SQ // QCH             # 512-wide chunks
    DSL = DM // P                # contraction slices for the projections
    KPC = QCH // P               # key tiles per chunk (4)

    nc = bacc.Bacc(None, target_bir_lowering=False)

    xT = nc.declare_dram_parameter("xT", [DM, SQ], bf16, isOutput=False)
    # packed projection weights: [Q01 | K01 | Q2K2] = 384 cols
    wqkv = nc.declare_dram_parameter("wqkv", [DM, 384], bf16, isOutput=False)
    # V weights for the direct [keys, z] projection, all 3 heads
    wv3 = nc.declare_dram_parameter("wv3", [DM, HPC * DH], bf16,
                                    isOutput=False)
    wo2 = nc.declare_dram_parameter("wo2", [P, DM], bf16, isOutput=False)
    wos = nc.declare_dram_parameter("wos", [DH, DM], bf16, isOutput=False)
    trimask = nc.declare_dram_parameter("trimask", [P, P], bf16, isOutput=False)
    ident_b = nc.declare_dram_parameter("ident_b", [P, P], bf16, isOutput=False)
    ones_z = nc.declare_dram_parameter("ones_z", [1, DH], f32r, isOutput=False)
    if use_biases:
        # per-partition bias columns for the 3 packed groups, plus the V
        # bias replicated across partitions (V bias varies along free dim)
        bqkv = nc.declare_dram_parameter("bqkv", [P, 3], f32, isOutput=False)
        bvrep = nc.declare_dram_parameter("bvrep", [P, HPC * DH], f32,
                                          isOutput=False)
    out = nc.declare_dram_parameter("out", [SQ, DM], bf16, isOutput=True)
    if DEBUG_DUMPS:
        dbg_kts = nc.declare_dram_parameter(
            "dbg_kts", [DH, SQ], bf16, isOutput=True)
        dbg_qts = nc.declare_dram_parameter(
            "dbg_qts", [DH, SQ], bf16, isOutput=True)
        dbg_qt2 = nc.declare_dram_parameter(
            "dbg_qt2", [P, SQ], bf16, isOutput=True)
        dbg_v = nc.declare_dram_parameter(
            "dbg_v", [P, HPC * (SQ // P) * (DH + 1)], bf16, isOutput=True)
        dbg_zn2 = nc.declare_dram_parameter(
            "dbg_zn2", [P, SQ], bf16, isOutput=True)

    with tile.TileContext(nc) as tc:
        with (
            tc.tile_pool(name="singles", bufs=1) as singles,
            tc.tile_pool(name="persist", bufs=1) as persist,
            tc.tile_pool(name="nrm_t", bufs=2) as nrm_t,
            tc.tile_pool(name="nrm_k", bufs=6) as nrm_k,
            tc.tile_pool(name="xT_pool", bufs=1) as xT_pool,
            tc.tile_pool(name="s_ps", bufs=2, space="PSUM") as s_ps,
            tc.tile_pool(name="zab_ps", bufs=1, space="PSUM") as zab_ps,
            tc.tile_pool(name="zc_ps", bufs=1, space="PSUM") as zc_ps,
            tc.tile_pool(name="fill_ps", bufs=1, space="PSUM") as fill_ps,
            tc.tile_pool(name="pt_sb", bufs=4) as pt_pool,
            tc.tile_pool(name="o_sb", bufs=6) as o_pool,
        ):
            # ---- constants / weights ----
            w_sb = singles.tile([P, DSL, 384], bf16, tag="wqkv")
            nc.sync.dma_start(w_sb[:], wqkv.rearrange("(o p) c -> p o c", p=P))
            wv_sb = singles.tile([P, DSL, HPC * DH], bf16, tag="wv3")
            nc.sync.dma_start(wv_sb[:], wv3.rearrange("(o p) c -> p o c", p=P))
            wo2_sb = singles.tile([P, DM], bf16)
            nc.sync.dma_start(wo2_sb[:], wo2[:])
            wos_sb = singles.tile([DH, DM], bf16)
            nc.sync.dma_start(wos_sb[:], wos[:])
            tri_sb = singles.tile([P, P], bf16)
            nc.sync.dma_start(tri_sb[:], trimask[:])
            idb_sb = singles.tile([P, P], bf16)
            nc.sync.dma_start(idb_sb[:], ident_b[:])
            ones_sb = singles.tile([1, DH], f32r)
            nc.sync.dma_start(ones_sb[:], ones_z[:])
            bias_sb = bv_sb = None
            if use_biases:
                bias_sb = singles.tile([P, 3], f32, tag="bias")
                nc.sync.dma_start(bias_sb[:], bqkv[:])
                bv_sb = singles.tile([P, HPC * DH], f32, tag="bvrep")
                nc.sync.dma_start(bv_sb[:], bvrep[:])

            # ---- persistent activations ----
            QT2 = persist.tile([P, SQ], bf16, tag="QT2")   # heads 0,1 stacked
            KT2 = persist.tile([P, SQ], bf16, tag="KT2")
            QTs = persist.tile([DH, SQ], bf16, tag="QTs")  # head 2
            KTs = persist.tile([DH, SQ], bf16, tag="KTs")
            V_sb = persist.tile([P, HPC, n_kt, DH + 1], bf16, tag="V")
            Zn2 = persist.tile([P, SQ], bf16, tag="Zn2")   # h0 rows 0-63, h1 64-127
            Zns = persist.tile([DH, SQ], bf16, tag="Zns")  # h2
            xT_sb = xT_pool.tile([P, DSL, SQ], bf16)
            # single-instance scratch for the row-sum reciprocal: rows past
            # the written ones are read by the 32x32 block transposes, so
            # they are zeroed once and the tiles reused in place
            r32g = persist.tile([32, QCH], f32, tag="r32g")
            rrTg = persist.tile([32, QCH], f32, tag="rrTg")

            nc.vector.memset(V_sb[:, :, :, DH:DH + 1], 1.0)
            nc.vector.memset(r32g[:], 1.0)
            nc.vector.memset(rrTg[:], 1.0)

            # stream xT in, chunk-major so early chunks land first
            for c in range(n_ch):
                nc.sync.dma_start(
                    xT_sb[:, :, c * QCH:(c + 1) * QCH],
                    xT.rearrange("(o p) c -> p o c", p=P)[
                        :, :, c * QCH:(c + 1) * QCH])

            # HAM warm-up: keep the PE MAC-busy while xT streams in, so the
            # first real matmuls run at 2.4 GHz instead of 1.2.
            wup = fill_ps.tile([P, QCH], f32, tag="fill", name="wup")
            for _ in range(40):
                nc.tensor.matmul(wup[:, 0:P], lhsT=idb_sb[:], rhs=idb_sb[:],
                                 start=True, stop=True)


            # ================= projection thunks =================
            # groups: 0=Q01, 1=K01, 2=Q2|K2; V is projected per key-tile
            # directly in [keys, z] layout (lhsT = xT slice).
            def proj_emit(g, c):
                """All matmuls + epilogue for group g, chunk c."""
                cs = slice(c * QCH, (c + 1) * QCH)
                ps = fill_ps.tile([P, QCH], f32, tag="fill", name="proj_ps")
                for o in range(DSL):
                    nc.tensor.matmul(
                        ps[:], lhsT=w_sb[:, o, g * P:(g + 1) * P],
                        rhs=xT_sb[:, o, cs],
                        start=(o == 0), stop=(o == DSL - 1))
                if use_biases:
                    def cp(dst, src, brow):
                        nc.vector.tensor_scalar(
                            dst, src, bias_sb[brow, g:g + 1], None, add)
                else:
                    def cp(dst, src, brow=None):
                        nc.vector.tensor_copy(dst, src)
                if g == 0:
                    cp(QT2[:, cs], ps[:], slice(0, P))
                elif g == 1:
                    cp(KT2[:, cs], ps[:], slice(0, P))
                else:
                    cp(QTs[:, cs], ps[0:DH], slice(0, DH))
                    hi = nrm_k.tile([P, QCH], bf16, tag="k2hi", name="k2hi")
                    cp(hi[DH:P, :], ps[DH:P], slice(DH, P))
                    nc.sync.dma_start(KTs[:, cs], hi[DH:P, :])

            def vproj_emit(kt):
                """V for all 3 heads of one key-tile: [128 keys, 192]."""
                ps = fill_ps.tile([P, QCH], f32, tag="fill",
                                  name="vproj_ps")[:, 0:HPC * DH]
                for o in range(DSL):
                    nc.tensor.matmul(
                        ps[:], lhsT=xT_sb[:, o, kt * P:(kt + 1) * P],
                        rhs=wv_sb[:, o, :],
                        start=(o == 0), stop=(o == DSL - 1))
                dst = V_sb[:, :, kt, 0:DH]
                src = ps.rearrange("p (h z) -> p h z", z=DH)
                if use_biases:
                    nc.vector.tensor_tensor(
                        dst, src, bv_sb.rearrange("p (h z) -> p h z", z=DH),
                        add)
                else:
                    nc.vector.tensor_copy(dst, src)

            def proj_thunks(c):
                for g in range(3):
                    yield (lambda g=g, c=c: proj_emit(g, c))
                for j in range(KPC):
                    yield (lambda kt=c * KPC + j: vproj_emit(kt))

            # ---- normalization helpers (DVE stage + deferred PE stage) ----
            def norm_stage1(za, zb, zc):
                """All-DVE: extract row sums, reciprocal via 32x32 block
                transposes, repack; copy Z tiles to SBUF."""
                staged = []
                for zacc in (za, zb, zc):
                    zsb = nrm_k.tile([DH, QCH], bf16, tag="zsb", name="zsb")
                    nc.vector.tensor_copy(zsb[:], zacc[0:DH, :])
                    nc.vector.tensor_copy(r32g[0:1, :], zacc[DH:DH + 1, :])
                    rT = nrm_t.tile([32, QCH], f32, tag="rT", name="rT")
                    nc.vector.transpose(rT[:], r32g[:])
                    nc.vector.reciprocal(
                        rrTg.rearrange("p (j c) -> p j c", c=32)[:, :, 0],
                        rT.rearrange("p (j c) -> p j c", c=32)[:, :, 0])
                    rr32 = nrm_t.tile([32, QCH], f32, tag="rr32", name="rr32")
                    nc.vector.transpose(rr32[:], rrTg[:])
                    rr = nrm_k.tile([1, QCH], f32r, tag="rr", name="rr_sb")
                    nc.vector.tensor_copy(rr[:], rr32[0:1, :])
                    staged.append((rr, zsb))
                return staged

            def norm_stage2(h, q0, staged):
                """PE rank-1 broadcast of 1/r, then one DVE multiply."""
                rr_sb, zsb = staged
                rrb = fill_ps.tile([P, QCH], f32, tag="fill",
                                   name="rrb")[0:DH]
                nc.tensor.matmul(rrb[:], lhsT=ones_sb[:],
                                 rhs=rr_sb[:], start=True, stop=True)
                if h == 0:
                    nc.vector.tensor_tensor(
                        Zn2[0:DH, q0:q0 + QCH], zsb[:], rrb[:], mult)
                elif h == 1:
                    t = nrm_k.tile([DH, QCH], bf16, tag="zn1", name="zn1")
                    nc.vector.tensor_tensor(t[:], zsb[:], rrb[:], mult)
                    nc.sync.dma_start(Zn2[DH:P, q0:q0 + QCH], t[:])
                else:
                    nc.vector.tensor_tensor(
                        Zns[:, q0:q0 + QCH], zsb[:], rrb[:], mult)

            # ===== flash: all heads interleaved, one 512-wide window loop ====
            SW = 2 * QCH            # psum score-slot width (tag "S")
            HD = DM // 2
            fills = []

            def oproj_thunks(w):
                """O-proj for window w as per-half-tile filler thunks:
                h01 packed (contraction 128) + h2 (contraction 64)."""
                thunks = []
                for tt in range(w * (QCH // P), (w + 1) * (QCH // P)):
                    osb = o_pool.tile([P, DM], bf16, tag="osb", name="osb")

                    def th(tt=tt, osb=osb, half=0):
                        po = fill_ps.tile([P, QCH], f32, tag="fill",
                                          name="po")[:, 0:HD]
                        hs = slice(half * HD, (half + 1) * HD)
                        nc.tensor.matmul(
                            po[:], lhsT=Zn2[:, tt * P:(tt + 1) * P],
                            rhs=wo2_sb[:, hs], start=True, stop=False)
                        nc.tensor.matmul(
                            po[:], lhsT=Zns[:, tt * P:(tt + 1) * P],
                            rhs=wos_sb[:, hs], start=False, stop=True)
                        nc.vector.tensor_copy(osb[:, hs], po[:])
                        if half == 1:
                            nc.sync.dma_start(out[tt * P:(tt + 1) * P, :],
                                              osb[:])

                    thunks.append(th)
                    thunks.append(lambda tt=tt, osb=osb, th=th: th(tt, osb, 1))
                return thunks

            # Window order: [1, 2, ..., n_ch-1, 0].  Window w needs proj
            # chunks 0..w, so chunks {0,1} run up front and chunk w+1 rides
            # window w's fills; the smallest window (0) runs last so the
            # final stage2/oproj tail is minimal.  stage2/oproj for the
            # previously processed window ride the current window's fills.
            worder = list(range(1, n_ch)) + [0]
            projected = {0, 1} if n_ch > 1 else {0}
            for c in sorted(projected):
                for th in proj_thunks(c):
                    th()

            staged = {}
            prev = None
            for wi, qs in enumerate(worder):
                q0 = qs * QCH
                nxt = worder[wi + 1] if wi + 1 < len(worder) else None
                if nxt is not None and nxt not in projected:
                    fills.extend(proj_thunks(nxt))
                    projected.add(nxt)
                if prev is not None:
                    for h in range(HPC):
                        fills.append(
                            (lambda h=h, q=prev * QCH, st=staged[(prev, h)]:
                             norm_stage2(h, q, st)))
                    fills.extend(oproj_thunks(prev))

                zab = zab_ps.tile([DH + 1, 2 * QCH], f32, tag="zab",
                                  name="zab")
                za = zab[:, 0:QCH]
                zb = zab[:, QCH:2 * QCH]
                zc = zc_ps.tile([DH + 1, QCH], f32, tag="zc", name="zc")
                nk = KPC * qs + KPC
                # --- heads 0,1: concurrent scores in two PE row groups ---
                for ki in range(nk):
                    vs = max(0, P * ki - q0)
                    ssc = s_ps.tile([P, SW], f32, tag="S", name="ssc")
                    nc.tensor.matmul(
                        ssc[:, vs:QCH],
                        lhsT=KT2[0:DH, ki * P:(ki + 1) * P],
                        rhs=QT2[0:DH, q0 + vs:q0 + QCH],
                        start=True, stop=True)
                    # head1 writes the full 512 so the joint exp below never
                    # reads PSUM bytes no matmul wrote (cols [QCH, QCH+vs)
                    # are computed-but-masked junk, never consumed)
                    nc.tensor.matmul(
                        ssc[:, QCH:2 * QCH],
                        lhsT=KT2[DH:P, ki * P:(ki + 1) * P],
                        rhs=QT2[DH:P, q0:q0 + QCH],
                        start=True, stop=True)
                    pt = pt_pool.tile([P, 2 * QCH], bf16, tag="PT",
                                      name="pt")
                    nc.scalar.activation(
                        pt[:, vs:], ssc[:, vs:2 * QCH], Exp, scale=0.125)
                    if ki >= KPC * qs:  # diagonal tile: mask both heads
                        blk = pt.rearrange(
                            "p (c w) -> p c w", c=2)[:, :, vs:vs + P]
                        nc.vector.tensor_tensor(
                            blk, blk,
                            tri_sb[:, None, :].to_broadcast(blk.shape),
                            mult)
                    nc.tensor.matmul(
                        za[:, vs:QCH], lhsT=V_sb[:, 0, ki, :],
                        rhs=pt[:, vs:QCH],
                        start=(ki == 0), stop=(ki == nk - 1))
                    nc.tensor.matmul(
                        zb[:, vs:QCH], lhsT=V_sb[:, 1, ki, :],
                        rhs=pt[:, QCH + vs:2 * QCH],
                        start=(ki == 0), stop=(ki == nk - 1))
                    if fills:
                        fills.pop(0)()
                # --- head 2: two key-tiles per exp call ---
                for kj in range(0, nk, 2):
                    vs0 = max(0, P * kj - q0)
                    vs1 = max(0, P * (kj + 1) - q0)
                    ssc = s_ps.tile([P, SW], f32, tag="S", name="ssc2")
                    nc.tensor.matmul(
                        ssc[:, vs0:QCH],
                        lhsT=KTs[:, kj * P:(kj + 1) * P],
                        rhs=QTs[:, q0 + vs0:q0 + QCH],
                        start=True, stop=True)
                    nc.tensor.matmul(
                        ssc[:, QCH:2 * QCH],
                        lhsT=KTs[:, (kj + 1) * P:(kj + 2) * P],
                        rhs=QTs[:, q0:q0 + QCH],
                        start=True, stop=True)
                    pt = pt_pool.tile([P, 2 * QCH], bf16, tag="PT2",
                                      name="pt2")
                    nc.scalar.activation(
                        pt[:, vs0:], ssc[:, vs0:2 * QCH], Exp, scale=0.125)
                    if kj >= KPC * qs:  # both tiles on the diagonal
                        nc.vector.tensor_tensor(
                            pt[:, vs0:vs0 + P], pt[:, vs0:vs0 + P],
                            tri_sb[:], mult)
                        nc.vector.tensor_tensor(
                            pt[:, QCH + vs1:QCH + vs1 + P],
                            pt[:, QCH + vs1:QCH + vs1 + P],
                            tri_sb[:], mult)
                    nc.tensor.matmul(
                        zc[:, vs0:QCH], lhsT=V_sb[:, 2, kj, :],
                        rhs=pt[:, vs0:QCH],
                        start=(kj == 0), stop=False)
                    nc.tensor.matmul(
                        zc[:, vs1:QCH], lhsT=V_sb[:, 2, kj + 1, :],
                        rhs=pt[:, QCH + vs1:2 * QCH],
                        start=False, stop=(kj + 1 == nk - 1))
                    if fills:
                        fills.pop(0)()
                staged[(qs, 0)], staged[(qs, 1)], staged[(qs, 2)] = (
                    norm_stage1(za, zb, zc))
                prev = qs
            while fills:
                fills.pop(0)()
            for h in range(HPC):
                norm_stage2(h, prev * QCH, staged[(prev, h)])
            for th in oproj_thunks(prev):
                th()
            if DEBUG_DUMPS:
                nc.sync.dma_start(dbg_kts[:], KTs[:])
                nc.sync.dma_start(dbg_qts[:], QTs[:])
                nc.sync.dma_start(dbg_qt2[:], QT2[:])
                nc.sync.dma_start(
                    dbg_v[:], V_sb.rearrange("p h k z -> p (h k z)"))
                nc.sync.dma_start(dbg_zn2[:], Zn2[:])

    nc.compile()
    return nc


def _prep_inputs(inputs, seq_len, use_biases):
    x = np.asarray(inputs["normalized_resid_pre"], dtype=np.float32)
    WQ = np.asarray(inputs["W_Q"], dtype=np.float32)
    WK = np.asarray(inputs["W_K"], dtype=np.float32)
    WV = np.asarray(inputs["W_V"], dtype=np.float32)
    WO = np.asarray(inputs["W_O"], dtype=np.float32)

    tri = np.triu(np.ones((P, P), np.float32)).astype(_BF)  # keep j >= p
    idb = np.eye(P, dtype=np.float32).astype(_BF)
    onz = np.ones((1, DH), np.float32)

    in_maps = []
    for c in range(NCORES):
        b, g = divmod(c, GROUPS)
        hs = slice(g * HPC, (g + 1) * HPC)
        wq = WQ[hs]   # [3, DM, DH]
        wk = WK[hs]
        wv = WV[hs]
        wo = WO[hs]   # [3, DH, DM]
        # packed groups: [Q01 | K01 | Q2K2] -> [DM, 384]
        wqkv = np.concatenate([
            wq[0], wq[1], wk[0], wk[1], wq[2], wk[2],
        ], axis=1)
        wv3 = np.concatenate([wv[0], wv[1], wv[2]], axis=1)
        m = {
            "xT": np.ascontiguousarray(x[b, :seq_len].T).astype(_BF),
            "wqkv": np.ascontiguousarray(wqkv).astype(_BF),
            "wv3": np.ascontiguousarray(wv3).astype(_BF),
            "wo2": np.ascontiguousarray(
                np.concatenate([wo[0], wo[1]], axis=0)).astype(_BF),
            "wos": np.ascontiguousarray(wo[2]).astype(_BF),
            "trimask": tri,
            "ident_b": idb,
            "ones_z": onz,
        }
        if use_biases:
            bq = np.asarray(inputs["b_Q"], np.float32)[hs]
            bk = np.asarray(inputs["b_K"], np.float32)[hs]
            bv = np.asarray(inputs["b_V"], np.float32)[hs]
            bias = np.zeros((P, 3), np.float32)
            bias[:, 0] = np.concatenate([bq[0], bq[1]])
            bias[:, 1] = np.concatenate([bk[0], bk[1]])
            bias[:, 2] = np.concatenate([bq[2], bk[2]])
            m["bqkv"] = bias
            m["bvrep"] = np.broadcast_to(
                bv.reshape(1, HPC * DH), (P, HPC * DH)).copy()
        in_maps.append(m)
    return in_maps


TRACE = False          # test.py can flip this to get exec_time_ns
last_result = None     # BassKernelResults of the most recent run


def kernel(seq_len=S, **inputs):
    global last_result
    from concourse.bass_utils import run_bass_kernel_spmd

    use_biases = any(
        np.any(np.asarray(inputs[k]) != 0) for k in ("b_Q", "b_K", "b_V"))

    key = (seq_len, use_biases)
    if key not in _cache:
        _cache[key] = _build(seq_len, use_biases)
    nc = _cache[key]

    in_maps = _prep_inputs(inputs, seq_len, use_biases)
    res = run_bass_kernel_spmd(nc, in_maps, core_ids=list(range(NCORES)),
                               trace=TRACE)
    last_result = res

    b_O = np.asarray(inputs["b_O"], dtype=np.float32)
    out = np.zeros((B, seq_len, DM), np.float32)
    for c in range(NCORES):
        b = c // GROUPS
        out[b] += np.asarray(res.results[c]["out"], dtype=np.float32)
    out += b_O[None, None, :]
    return out


# revision 34
# speedup vs baseline: 1.2726x; 1.2726x over previous
"""Causal multi-head attention on 8 Trainium2 NeuronCores.

Problem: B=2, S=4096, D_MODEL=768, H=12, D_HEAD=64, fp32 I/O.

Sharding: (batch, head-group) -> core.  Cores 0-3 take batch 0, cores 4-7
take batch 1; each core computes 3 of the 12 heads for its batch and emits a
partial output [S, D_MODEL] (its heads' contribution to the W_O contraction)
in bf16.  The host sums the 4 partials per batch and adds b_O.

v2 design (vs v1): the scalar engine's EXP throughput (1 elem/cycle/lane) is
the fundamental floor (~190us/core), so the kernel keeps ACT dense from t~5us
and strips everything else off it:
  1. QKV projections are interleaved INTO the flash window loop as PE filler
     thunks (proj chunk c+2 runs during window c), so exp starts immediately
     after chunk 0 is projected.  Projection weights are packed into 5 column
     groups [Q01|K01|V01|Q2K2|V2] so 4 of 5 matmuls use the full 128-wide PE.
     Epilogue copies run on DVE (not ACT).  K2's partition shift (rows 64-127
     -> 0-63) rides a small SBUF->SBUF DMA.
  2. V is transposed to [keys, z] layout by the DMA xbar (dma transpose),
     costing zero PE/DVE cycles; a ones column gives softmax row sums for
     free in the AV matmuls (PSUM row 64).
  3. Head2 exp is batched over 2 key-tiles per call ([128,1024] activations
     everywhere), halving ACT per-instruction overhead.
  4. O-projection packs heads 0,1 as one contraction-128 matmul: Zn for h1 is
     partition-shifted to rows 64-127 by DMA, so per row-tile the projection
     is 2 matmuls (h01 + h2) instead of 3.
  5. Output partials are written in bf16 (halves output DMA); the host sums
     the 4 partials per batch in fp32.
  6. Softmax row-sum normalization (DVE transpose + reciprocal) is batched
     across the 3 heads per window; the 1/r broadcast stays a rank-1 PE
     matmul emitted as a filler so the PE never stalls long enough to
     re-throttle its clock (HAM drops 2.4->1.2 GHz after ~3.4us idle).
"""

import numpy as np
import ml_dtypes

B, S, DM, H, DH = 2, 4096, 768, 12, 64
NCORES = 8
GROUPS = 4                  # head-groups per batch
HPC = H // GROUPS           # heads per core = 3
P = 128
QCH = 512                   # psum bank width (fp32)

_BF = ml_dtypes.bfloat16

_cache = {}
DEBUG_DUMPS = False


def _build(seq_len, use_biases):
    import concourse.bacc as bacc
    import concourse.mybir as mybir
    import concourse.tile as tile

    f32 = mybir.dt.float32
    f32r = mybir.dt.float32r
    bf16 = mybir.dt.bfloat16
    Exp = mybir.ActivationFunctionType.Exp
    mult = mybir.AluOpType.mult
    add = mybir.AluOpType.add

    SQ = seq_len
    n_kt = SQ // P               # k tiles
    n_tt = SQ // P               # output row tiles
    n_ch = SQ // QCH             # 512-wide chunks
    DSL = DM // P                # contraction slices for the projections
    KPC = QCH // P               # key tiles per chunk (4)

    nc = bacc.Bacc(None, target_bir_lowering=False)

    xT = nc.declare_dram_parameter("xT", [DM, SQ], bf16, isOutput=False)
    # packed projection weights: [Q01 | K01 | Q2K2] = 384 cols
    wqkv = nc.declare_dram_parameter("wqkv", [DM, 384], bf16, isOutput=False)
    # V weights for the direct [keys, z] projection, all 3 heads
    wv3 = nc.declare_dram_parameter("wv3", [DM, HPC * DH], bf16,
                                    isOutput=False)
    wo2 = nc.declare_dram_parameter("wo2", [P, DM], bf16, isOutput=False)
    wos = nc.declare_dram_parameter("wos", [DH, DM], bf16, isOutput=False)
    trimask = nc.declare_dram_parameter("trimask", [P, P], bf16, isOutput=False)
    ident_b = nc.declare_dram_parameter("ident_b", [P, P], bf16, isOutput=False)
    ones_z = nc.declare_dram_parameter("ones_z", [1, DH], f32r, isOutput=False)
    if use_biases:
        # per-partition bias columns for the 3 packed groups, plus the V
        # bias replicated across partitions (V bias varies along free dim)
        bqkv = nc.declare_dram_parameter("bqkv", [P, 3], f32, isOutput=False)
        bvrep = nc.declare_dram_parameter("bvrep", [P, HPC * DH], f32,
                                          isOutput=False)
    out = nc.declare_dram_parameter("out", [SQ, DM], bf16, isOutput=True)
    if DEBUG_DUMPS:
        dbg_kts = nc.declare_dram_parameter(
            "dbg_kts", [DH, SQ], bf16, isOutput=True)
        dbg_qts = nc.declare_dram_parameter(
            "dbg_qts", [DH, SQ], bf16, isOutput=True)
        dbg_qt2 = nc.declare_dram_parameter(
            "dbg_qt2", [P, SQ], bf16, isOutput=True)
        dbg_v = nc.declare_dram_parameter(
            "dbg_v", [P, HPC * (SQ // P) * (DH + 1)], bf16, isOutput=True)
        dbg_zn2 = nc.declare_dram_parameter(
            "dbg_zn2", [P, SQ], bf16, isOutput=True)

    with tile.TileContext(nc) as tc:
        with (
            tc.tile_pool(name="singles", bufs=1) as singles,
            tc.tile_pool(name="persist", bufs=1) as persist,
            tc.tile_pool(name="nrm_t", bufs=2) as nrm_t,
            tc.tile_pool(name="nrm_k", bufs=6) as nrm_k,
            tc.tile_pool(name="xT_pool", bufs=1) as xT_pool,
            tc.tile_pool(name="s_ps", bufs=2, space="PSUM") as s_ps,
            tc.tile_pool(name="zab_ps", bufs=1, space="PSUM") as zab_ps,
            tc.tile_pool(name="zc_ps", bufs=1, space="PSUM") as zc_ps,
            tc.tile_pool(name="fill_ps", bufs=1, space="PSUM") as fill_ps,
            tc.tile_pool(name="pt_sb", bufs=4) as pt_pool,
            tc.tile_pool(name="o_sb", bufs=6) as o_pool,
        ):
            # ---- constants / weights ----
            w_sb = singles.tile([P, DSL, 384], bf16, tag="wqkv")
            nc.sync.dma_start(w_sb[:], wqkv.rearrange("(o p) c -> p o c", p=P))
            wv_sb = singles.tile([P, DSL, HPC * DH], bf16, tag="wv3")
            nc.sync.dma_start(wv_sb[:], wv3.rearrange("(o p) c -> p o c", p=P))
            wo2_sb = singles.tile([P, DM], bf16)
            nc.sync.dma_start(wo2_sb[:], wo2[:])
            wos_sb = singles.tile([DH, DM], bf16)
            nc.sync.dma_start(wos_sb[:], wos[:])
            tri_sb = singles.tile([P, P], bf16)
            nc.sync.dma_start(tri_sb[:], trimask[:])
            idb_sb = singles.tile([P, P], bf16)
            nc.sync.dma_start(idb_sb[:], ident_b[:])
            ones_sb = singles.tile([1, DH], f32r)
            nc.sync.dma_start(ones_sb[:], ones_z[:])
            bias_sb = bv_sb = None
            if use_biases:
                bias_sb = singles.tile([P, 3], f32, tag="bias")
                nc.sync.dma_start(bias_sb[:], bqkv[:])
                bv_sb = singles.tile([P, HPC * DH], f32, tag="bvrep")
                nc.sync.dma_start(bv_sb[:], bvrep[:])

            # ---- persistent activations ----
            QT2 = persist.tile([P, SQ], bf16, tag="QT2")   # heads 0,1 stacked
            KT2 = persist.tile([P, SQ], bf16, tag="KT2")
            QTs = persist.tile([DH, SQ], bf16, tag="QTs")  # head 2
            KTs = persist.tile([DH, SQ], bf16, tag="KTs")
            V_sb = persist.tile([P, HPC, n_kt, DH + 1], bf16, tag="V")
            Zn2 = persist.tile([P, SQ], bf16, tag="Zn2")   # h0 rows 0-63, h1 64-127
            Zns = persist.tile([DH, SQ], bf16, tag="Zns")  # h2
            xT_sb = xT_pool.tile([P, DSL, SQ], bf16)
            # single-instance scratch for the row-sum reciprocal: rows past
            # the written ones are read by the 32x32 block transposes, so
            # they are zeroed once and the tiles reused in place
            r32g = persist.tile([32, QCH], f32, tag="r32g")
            rrTg = persist.tile([32, QCH], f32, tag="rrTg")

            nc.vector.memset(V_sb[:, :, :, DH:DH + 1], 1.0)
            nc.vector.memset(r32g[:], 1.0)
            nc.vector.memset(rrTg[:], 1.0)

            # stream xT in, chunk-major so early chunks land first
            for c in range(n_ch):
                nc.sync.dma_start(
                    xT_sb[:, :, c * QCH:(c + 1) * QCH],
                    xT.rearrange("(o p) c -> p o c", p=P)[
                        :, :, c * QCH:(c + 1) * QCH])

            # HAM warm-up: keep the PE MAC-busy while xT streams in, so the
            # first real matmuls run at 2.4 GHz instead of 1.2.
            wup = fill_ps.tile([P, QCH], f32, tag="fill", name="wup")
            for _ in range(40):
                nc.tensor.matmul(wup[:, 0:P], lhsT=idb_sb[:], rhs=idb_sb[:],
                                 start=True, stop=True)


            # ================= projection thunks =================
            # groups: 0=Q01, 1=K01, 2=Q2|K2; V is projected per key-tile
            # directly in [keys, z] layout (lhsT = xT slice).
            def proj_emit(g, c):
                """All matmuls + epilogue for group g, chunk c."""
                cs = slice(c * QCH, (c + 1) * QCH)
                ps = fill_ps.tile([P, QCH], f32, tag="fill", name="proj_ps")
                for o in range(DSL):
                    nc.tensor.matmul(
                        ps[:], lhsT=w_sb[:, o, g * P:(g + 1) * P],
                        rhs=xT_sb[:, o, cs],
                        start=(o == 0), stop=(o == DSL - 1))
                if use_biases:
                    def cp(dst, src, brow):
                        nc.vector.tensor_scalar(
                            dst, src, bias_sb[brow, g:g + 1], None, add)
                else:
                    def cp(dst, src, brow=None):
                        nc.vector.tensor_copy(dst, src)
                if g == 0:
                    cp(QT2[:, cs], ps[:], slice(0, P))
                elif g == 1:
                    cp(KT2[:, cs], ps[:], slice(0, P))
                else:
                    cp(QTs[:, cs], ps[0:DH], slice(0, DH))
                    hi = nrm_k.tile([P, QCH], bf16, tag="k2hi", name="k2hi")
                    cp(hi[DH:P, :], ps[DH:P], slice(DH, P))
                    nc.sync.dma_start(KTs[:, cs], hi[DH:P, :])

            def vproj_emit(kt):
                """V for all 3 heads of one key-tile: [128 keys, 192]."""
                ps = fill_ps.tile([P, QCH], f32, tag="fill",
                                  name="vproj_ps")[:, 0:HPC * DH]
                for o in range(DSL):
                    nc.tensor.matmul(
                        ps[:], lhsT=xT_sb[:, o, kt * P:(kt + 1) * P],
                        rhs=wv_sb[:, o, :],
                        start=(o == 0), stop=(o == DSL - 1))
                dst = V_sb[:, :, kt, 0:DH]
                src = ps.rearrange("p (h z) -> p h z", z=DH)
                if use_biases:
                    nc.vector.tensor_tensor(
                        dst, src, bv_sb.rearrange("p (h z) -> p h z", z=DH),
                        add)
                else:
                    nc.vector.tensor_copy(dst, src)

            def proj_thunks(c):
                for g in range(3):
                    yield (lambda g=g, c=c: proj_emit(g, c))
                for j in range(KPC):
                    yield (lambda kt=c * KPC + j: vproj_emit(kt))

            # ---- normalization helpers (DVE stage + deferred PE stage) ----
            def norm_stage1(za, zb, zc):
                """All-DVE: extract row sums, reciprocal via 32x32 block
                transposes, repack; copy Z tiles to SBUF."""
                staged = []
                for zacc in (za, zb, zc):
                    zsb = nrm_k.tile([DH, QCH], bf16, tag="zsb", name="zsb")
                    nc.vector.tensor_copy(zsb[:], zacc[0:DH, :])
                    nc.vector.tensor_copy(r32g[0:1, :], zacc[DH:DH + 1, :])
                    rT = nrm_t.tile([32, QCH], f32, tag="rT", name="rT")
                    nc.vector.transpose(rT[:], r32g[:])
                    nc.vector.reciprocal(
                        rrTg.rearrange("p (j c) -> p j c", c=32)[:, :, 0],
                        rT.rearrange("p (j c) -> p j c", c=32)[:, :, 0])
                    rr32 = nrm_t.tile([32, QCH], f32, tag="rr32", name="rr32")
                    nc.vector.transpose(rr32[:], rrTg[:])
                    rr = nrm_k.tile([1, QCH], f32r, tag="rr", name="rr_sb")
                    nc.vector.tensor_copy(rr[:], rr32[0:1, :])
                    staged.append((rr, zsb))
                return staged

            def norm_stage2(h, q0, staged):
                """PE rank-1 broadcast of 1/r, then one DVE multiply."""
                rr_sb, zsb = staged
                rrb = fill_ps.tile([P, QCH], f32, tag="fill",
                                   name="rrb")[0:DH]
                nc.tensor.matmul(rrb[:], lhsT=ones_sb[:],
                                 rhs=rr_sb[:], start=True, stop=True)
                if h == 0:
                    nc.vector.tensor_tensor(
                        Zn2[0:DH, q0:q0 + QCH], zsb[:], rrb[:], mult)
                elif h == 1:
                    t = nrm_k.tile([DH, QCH], bf16, tag="zn1", name="zn1")
                    nc.vector.tensor_tensor(t[:], zsb[:], rrb[:], mult)
                    nc.sync.dma_start(Zn2[DH:P, q0:q0 + QCH], t[:])
                else:
                    nc.vector.tensor_tensor(
                        Zns[:, q0:q0 + QCH], zsb[:], rrb[:], mult)

            # ===== flash: all heads interleaved, one 512-wide window loop ====
            SW = 2 * QCH            # psum score-slot width (tag "S")
            HD = DM // 2
            fills = []

            def oproj_thunks(w):
                """O-proj for window w as per-half-tile filler thunks:
                h01 packed (contraction 128) + h2 (contraction 64)."""
                thunks = []
                for tt in range(w * (QCH // P), (w + 1) * (QCH // P)):
                    osb = o_pool.tile([P, DM], bf16, tag="osb", name="osb")

                    def th(tt=tt, osb=osb, half=0):
                        po = fill_ps.tile([P, QCH], f32, tag="fill",
                                          name="po")[:, 0:HD]
                        hs = slice(half * HD, (half + 1) * HD)
                        nc.tensor.matmul(
                            po[:], lhsT=Zn2[:, tt * P:(tt + 1) * P],
                            rhs=wo2_sb[:, hs], start=True, stop=False)
                        nc.tensor.matmul(
                            po[:], lhsT=Zns[:, tt * P:(tt + 1) * P],
                            rhs=wos_sb[:, hs], start=False, stop=True)
                        nc.vector.tensor_copy(osb[:, hs], po[:])
                        if half == 1:
                            nc.sync.dma_start(out[tt * P:(tt + 1) * P, :],
                                              osb[:])

                    thunks.append(th)
                    thunks.append(lambda tt=tt, osb=osb, th=th: th(tt, osb, 1))
                return thunks

            # Ascending windows; proj chunks {0,1} up front, chunk qs+2
            # rides window qs's fills.  stage2/oproj for window qs-1 also
            # ride window qs, with oproj queued after proj so its stage2 /
            # Zn-shift-DMA inputs are settled by the time it pops.
            for c in range(min(2, n_ch)):
                for th in proj_thunks(c):
                    th()

            staged = {}
            prev = None
            for qs in range(n_ch):
                q0 = qs * QCH
                if prev is not None:
                    for h in range(HPC):
                        fills.append(
                            (lambda h=h, q=prev * QCH, st=staged[(prev, h)]:
                             norm_stage2(h, q, st)))
                if qs + 2 < n_ch:
                    fills.extend(proj_thunks(qs + 2))
                if prev is not None:
                    fills.extend(oproj_thunks(prev))

                zab = zab_ps.tile([DH + 1, 2 * QCH], f32, tag="zab",
                                  name="zab")
                za = zab[:, 0:QCH]
                zb = zab[:, QCH:2 * QCH]
                zc = zc_ps.tile([DH + 1, QCH], f32, tag="zc", name="zc")
                nk = KPC * qs + KPC
                # --- heads 0,1: concurrent scores in two PE row groups ---
                for ki in range(nk):
                    vs = max(0, P * ki - q0)
                    ssc = s_ps.tile([P, SW], f32, tag="S", name="ssc")
                    nc.tensor.matmul(
                        ssc[:, vs:QCH],
                        lhsT=KT2[0:DH, ki * P:(ki + 1) * P],
                        rhs=QT2[0:DH, q0 + vs:q0 + QCH],
                        start=True, stop=True)
                    # head1 writes the full 512 so the joint exp below never
                    # reads PSUM bytes no matmul wrote (cols [QCH, QCH+vs)
                    # are computed-but-masked junk, never consumed)
                    nc.tensor.matmul(
                        ssc[:, QCH:2 * QCH],
                        lhsT=KT2[DH:P, ki * P:(ki + 1) * P],
                        rhs=QT2[DH:P, q0:q0 + QCH],
                        start=True, stop=True)
                    pt = pt_pool.tile([P, 2 * QCH], bf16, tag="PT",
                                      name="pt")
                    nc.scalar.activation(
                        pt[:, vs:], ssc[:, vs:2 * QCH], Exp, scale=0.125)
                    if ki >= KPC * qs:  # diagonal tile: mask both heads
                        blk = pt.rearrange(
                            "p (c w) -> p c w", c=2)[:, :, vs:vs + P]
                        nc.vector.tensor_tensor(
                            blk, blk,
                            tri_sb[:, None, :].to_broadcast(blk.shape),
                            mult)
                    nc.tensor.matmul(
                        za[:, vs:QCH], lhsT=V_sb[:, 0, ki, :],
                        rhs=pt[:, vs:QCH],
                        start=(ki == 0), stop=(ki == nk - 1))
                    nc.tensor.matmul(
                        zb[:, vs:QCH], lhsT=V_sb[:, 1, ki, :],
                        rhs=pt[:, QCH + vs:2 * QCH],
                        start=(ki == 0), stop=(ki == nk - 1))
                    if fills:
                        fills.pop(0)()
                # --- head 2: two key-tiles per exp call ---
                for kj in range(0, nk, 2):
                    vs0 = max(0, P * kj - q0)
                    vs1 = max(0, P * (kj + 1) - q0)
                    ssc = s_ps.tile([P, SW], f32, tag="S", name="ssc2")
                    nc.tensor.matmul(
                        ssc[:, vs0:QCH],
                        lhsT=KTs[:, kj * P:(kj + 1) * P],
                        rhs=QTs[:, q0 + vs0:q0 + QCH],
                        start=True, stop=True)
                    nc.tensor.matmul(
                        ssc[:, QCH:2 * QCH],
                        lhsT=KTs[:, (kj + 1) * P:(kj + 2) * P],
                        rhs=QTs[:, q0:q0 + QCH],
                        start=True, stop=True)
                    pt = pt_pool.tile([P, 2 * QCH], bf16, tag="PT2",
                                      name="pt2")
                    nc.scalar.activation(
                        pt[:, vs0:], ssc[:, vs0:2 * QCH], Exp, scale=0.125)
                    if kj >= KPC * qs:  # both tiles on the diagonal
                        nc.vector.tensor_tensor(
                            pt[:, vs0:vs0 + P], pt[:, vs0:vs0 + P],
                            tri_sb[:], mult)
                        nc.vector.tensor_tensor(
                            pt[:, QCH + vs1:QCH + vs1 + P],
                            pt[:, QCH + vs1:QCH + vs1 + P],
                            tri_sb[:], mult)
                    nc.tensor.matmul(
                        zc[:, vs0:QCH], lhsT=V_sb[:, 2, kj, :],
                        rhs=pt[:, vs0:QCH],
                        start=(kj == 0), stop=False)
                    nc.tensor.matmul(
                        zc[:, vs1:QCH], lhsT=V_sb[:, 2, kj + 1, :],
                        rhs=pt[:, QCH + vs1:2 * QCH],
                        start=False, stop=(kj + 1 == nk - 1))
                    if fills:
                        fills.pop(0)()
                staged[(qs, 0)], staged[(qs, 1)], staged[(qs, 2)] = (
                    norm_stage1(za, zb, zc))
                prev = qs
            while fills:
                fills.pop(0)()
            for h in range(HPC):
                norm_stage2(h, prev * QCH, staged[(prev, h)])
            for th in oproj_thunks(prev):
                th()
            if DEBUG_DUMPS:
                nc.sync.dma_start(dbg_kts[:], KTs[:])
                nc.sync.dma_start(dbg_qts[:], QTs[:])
                nc.sync.dma_start(dbg_qt2[:], QT2[:])
                nc.sync.dma_start(
                    dbg_v[:], V_sb.rearrange("p h k z -> p (h k z)"))
                nc.sync.dma_start(dbg_zn2[:], Zn2[:])

    nc.compile()
    return nc


def _prep_inputs(inputs, seq_len, use_biases):
    x = np.asarray(inputs["normalized_resid_pre"], dtype=np.float32)
    WQ = np.asarray(inputs["W_Q"], dtype=np.float32)
    WK = np.asarray(inputs["W_K"], dtype=np.float32)
    WV = np.asarray(inputs["W_V"], dtype=np.float32)
    WO = np.asarray(inputs["W_O"], dtype=np.float32)

    tri = np.triu(np.ones((P, P), np.float32)).astype(_BF)  # keep j >= p
    idb = np.eye(P, dtype=np.float32).astype(_BF)
    onz = np.ones((1, DH), np.float32)

    in_maps = []
    for c in range(NCORES):
        b, g = divmod(c, GROUPS)
        hs = slice(g * HPC, (g + 1) * HPC)
        wq = WQ[hs]   # [3, DM, DH]
        wk = WK[hs]
        wv = WV[hs]
        wo = WO[hs]   # [3, DH, DM]
        # packed groups: [Q01 | K01 | Q2K2] -> [DM, 384]
        wqkv = np.concatenate([
            wq[0], wq[1], wk[0], wk[1], wq[2], wk[2],
        ], axis=1)
        wv3 = np.concatenate([wv[0], wv[1], wv[2]], axis=1)
        m = {
            "xT": np.ascontiguousarray(x[b, :seq_len].T).astype(_BF),
            "wqkv": np.ascontiguousarray(wqkv).astype(_BF),
            "wv3": np.ascontiguousarray(wv3).astype(_BF),
            "wo2": np.ascontiguousarray(
                np.concatenate([wo[0], wo[1]], axis=0)).astype(_BF),
            "wos": np.ascontiguousarray(wo[2]).astype(_BF),
            "trimask": tri,
            "ident_b": idb,
            "ones_z": onz,
        }
        if use_biases:
            bq = np.asarray(inputs["b_Q"], np.float32)[hs]
            bk = np.asarray(inputs["b_K"], np.float32)[hs]
            bv = np.asarray(inputs["b_V"], np.float32)[hs]
            bias = np.zeros((P, 3), np.float32)
            bias[:, 0] = np.concatenate([bq[0], bq[1]])
            bias[:, 1] = np.concatenate([bk[0], bk[1]])
            bias[:, 2] = np.concatenate([bq[2], bk[2]])
            m["bqkv"] = bias
            m["bvrep"] = np.broadcast_to(
                bv.reshape(1, HPC * DH), (P, HPC * DH)).copy()
        in_maps.append(m)
    return in_maps


TRACE = False          # test.py can flip this to get exec_time_ns
last_result = None     # BassKernelResults of the most recent run


def kernel(seq_len=S, **inputs):
    global last_result
    from concourse.bass_utils import run_bass_kernel_spmd

    use_biases = any(
        np.any(np.asarray(inputs[k]) != 0) for k in ("b_Q", "b_K", "b_V"))

    key = (seq_len, use_biases)
    if key not in _cache:
        _cache[key] = _build(seq_len, use_biases)
    nc = _cache[key]

    in_maps = _prep_inputs(inputs, seq_len, use_biases)
    res = run_bass_kernel_spmd(nc, in_maps, core_ids=list(range(NCORES)),
                               trace=TRACE)
    last_result = res

    b_O = np.asarray(inputs["b_O"], dtype=np.float32)
    out = np.zeros((B, seq_len, DM), np.float32)
    for c in range(NCORES):
        b = c // GROUPS
        out[b] += np.asarray(res.results[c]["out"], dtype=np.float32)
    out += b_O[None, None, :]
    return out
